# revision 6
# baseline (speedup 1.0000x reference)
"""Trainium2 Bass kernel for EntropyRegularizationLoss.

loss = mean_c H(softmax(G[c,:])) + H(softmax(G.sum(0)))  where
G = normalize(P) @ normalize(P).T,  P = prototypes [8192, 512] f32.

Distribution: 8 cores; core i receives P rotated by i*1024 rows so one
identical SPMD program always works on "rows 0:1024" as its Gram row
block. Row rotation permutes rows AND columns of G, which leaves both
the row entropies (row-local) and the colsum entropy (permutation
invariant) unchanged. One AllGather of [1024 rowsums | local H sum]
combines the cores.

Key algebraic moves:
  - normalization is folded into the operands, so the Gram comes out of
    the PE already normalized;
  - logits are cosine sims in [-1, 1] -> exp needs no max subtraction;
  - colsum of the (symmetric) full Gram == rowsum, and rowsum_c =
    p_c . S with S = sum_d p_d: a matvec instead of a 67M-elem pass;
  - ACT exp accumulates Z = sum e^x in-instruction (accum_out);
  - DVE tensor_tensor_reduce fuses w = x*e^x with W = sum w.
"""
import sys, json

sys.path.insert(0, "/opt/trn_rl_repo")

import numpy as np

import concourse.bass as bass
import concourse.tile as tile
from concourse import mybir
from concourse.bass_utils import run_bass_kernel_spmd

AF = mybir.ActivationFunctionType
ALU = mybir.AluOpType
F32 = mybir.dt.float32
BF16 = mybir.dt.bfloat16

N_CORES = 8
C, E = 8192, 512
RPC = C // N_CORES          # rows per core = 1024
CBLK = RPC // 128           # c-blocks per core = 8
KCH = E // 128              # contraction chunks = 4
NT = C // 128               # natural tiles = 64
DG = 1024                   # d-group width (2 PSUM banks)
NDG = C // DG               # d-groups = 4


def _split_sync_waits(bir: bytes, max_waits: int = 1) -> bytes:
    """walrus codegen rejects instructions with more than a few sem waits;
    split excess waits into chained Drain stubs on the same engine."""
    m = json.loads(bir)
    uid = [0]
    for fn in m["functions"]:
        for bb in fn["blocks"]:
            out_insts = []
            for inst in bb["instructions"]:
                si = inst.get("sync_info")
                ow = si.get("on_wait") if si else None
                if ow and len(ow) > max_waits:
                    chunks = [ow[i:i + max_waits] for i in range(0, len(ow), max_waits)]
                    for ch in chunks[:-1]:
                        uid[0] += 1
                        out_insts.append({
                            "debug": inst.get("debug"),
                            "engine": inst["engine"],
                            "ins": [],
                            "is_reset_sema": False,
                            "name": f"I-waitsplit-{uid[0]}",
                            "opcode": "Drain",
                            "outs": [],
                            "sync_info": {"on_update": [], "on_wait": ch},
                        })
                    si["on_wait"] = chunks[-1]
                out_insts.append(inst)
            bb["instructions"] = out_insts
    return json.dumps(m).encode()


def build_program(n_iters: int = 1):
    import ml_dtypes

    nc = bass.Bass("TRN2", target_bir_lowering=False, debug=False,
                   num_devices=N_CORES)
    proto = nc.dram_tensor("prototypes", [C, E], F32, kind="ExternalInput").ap()
    loss_out = nc.dram_tensor("loss", [1], F32, kind="ExternalOutput").ap()

    eye_bf = nc.inline_tensor(np.eye(128, dtype=ml_dtypes.bfloat16), name="eye_bf").ap()
    eye_f32 = nc.inline_tensor(np.eye(128, dtype=np.float32), name="eye_f32").ap()

    cc_in = nc.dram_tensor("cc_in", [RPC + 1], F32)
    cc_out = nc.dram_tensor("cc_out", [N_CORES * (RPC + 1)], F32,
                            addr_space="Shared")

    with tile.TileContext(nc) as tc:
        with tc.tile_pool(name="singles", bufs=1) as singles:
            ones_col = singles.tile([128, 1], F32)   # lhsT for partition sums
            nc.vector.memset(ones_col, 1.0)
            ones_row = singles.tile([1, 128], F32)   # lhsT for partition bcast
            nc.vector.memset(ones_row, 1.0)
            eye_b = singles.tile([128, 128], BF16)
            nc.sync.dma_start(eye_b[:], eye_bf[:])
            eye_f = singles.tile([128, 128], F32)
            nc.sync.dma_start(eye_f[:], eye_f32[:])

            for _ in range(n_iters):
                _one_iter(nc, tc, proto, loss_out, cc_in, cc_out,
                          ones_col, ones_row, eye_b, eye_f)

    orig = nc.to_json_bytes
    nc.to_json_bytes = lambda: _split_sync_waits(orig())
    return nc


def _one_iter(nc, tc, proto, loss_out, cc_in, cc_out,
              ones_col, ones_row, eye_b, eye_f):
    with (
        tc.tile_pool(name="persist", bufs=1) as persist,
        tc.tile_pool(name="nat", bufs=4) as natp,
        tc.tile_pool(name="scaled", bufs=4) as scp,
        tc.tile_pool(name="scratch", bufs=2) as scr,
        tc.tile_pool(name="small", bufs=2) as small,
    ):
        # protoT[k]: [128, 8192] bf16, k-th 128-slice of E on partitions
        protoT = [persist.tile([128, C], BF16, tag=f"pt{k}", name=f"pt{k}") for k in range(KCH)]
        normsq = persist.tile([128, NT], F32, tag="normsq")
        invn = persist.tile([128, NT], F32, tag="invn")
        sparts = [persist.tile([128, NT // 4], F32, tag=f"sp{k}", name=f"sp{k}") for k in range(KCH)]
        zparts = persist.tile([128, CBLK * NDG], F32, tag="zparts")
        wparts = persist.tile([128, CBLK * NDG], F32, tag="wparts")
        rowsums = persist.tile([128, CBLK], F32, tag="rowsums")
        s_bf = persist.tile([128, KCH], BF16, tag="s_bf")

        # Wavefront over chunks of 16 natural tiles: load+normalize+transpose
        # chunk c, then immediately emit the Gram d-groups that chunk c
        # unlocked (dg = 2c, 2c+1 vs every cb; the lhsT c-blocks all live in
        # chunk 0's columns). This keeps PE's serial instruction stream dense:
        # transposes(c) | grams(c) | transposes(c+1) | grams(c+1) | ...
        # psT (4 banks) and psG (4 banks, nested) get disjoint PSUM banks.
        CHUNK = 16
        with (
            tc.tile_pool(name="psumT", bufs=1, space="PSUM") as psT,
            tc.tile_pool(name="psumG", bufs=2, space="PSUM") as psG,
            tc.tile_pool(name="ew", bufs=3) as ewp,
        ):
            for chunk in range(NT // CHUNK):
                nats = []
                for j in range(CHUNK):
                    t = chunk * CHUNK + j
                    nat = natp.tile([128, E], F32, tag=f"nat{j % 4}")
                    nc.sync.dma_start(nat[:], proto[t * 128:(t + 1) * 128, :])
                    sq = scr.tile([128, E], F32, tag="sq")
                    nc.vector.scalar_tensor_tensor(
                        out=sq[:], in0=nat[:], scalar=1.0, in1=nat[:],
                        op0=ALU.mult, op1=ALU.mult,
                        accum_out=normsq[:, t:t + 1])
                    nats.append(nat)
                c16 = slice(chunk * CHUNK, (chunk + 1) * CHUNK)
                snorm = small.tile([128, CHUNK], F32, tag="snorm")
                nc.scalar.activation(snorm[:], normsq[:, c16], AF.Sqrt)
                r0 = small.tile([128, CHUNK], F32, tag="r0")
                nc.vector.reciprocal(r0[:], snorm[:])
                # one Newton step for rsqrt: y = r0*(1.5 - 0.5*x*r0^2)
                t1 = small.tile([128, CHUNK], F32, tag="t1")
                nc.vector.tensor_tensor(t1[:], r0[:], r0[:], ALU.mult)
                nc.vector.tensor_tensor(t1[:], t1[:], normsq[:, c16], ALU.mult)
                nc.vector.tensor_scalar(out=t1[:], in0=t1[:], scalar1=-0.5,
                                        scalar2=1.5, op0=ALU.mult, op1=ALU.add)
                nc.vector.tensor_tensor(invn[:, c16], r0[:], t1[:], ALU.mult)

                for j in range(CHUNK):
                    t = chunk * CHUNK + j
                    sc = scp.tile([128, E], BF16, tag=f"sc{j % 4}")
                    nc.gpsimd.tensor_scalar(out=sc[:], in0=nats[j][:],
                                            scalar1=invn[:, t:t + 1],
                                            scalar2=None, op0=ALU.mult)
                    nats[j] = sc
                # transpose groups of 4 tiles -> protoT[k][:, grp*512:+512]
                for g4 in range(CHUNK // 4):
                    grp = chunk * 4 + g4
                    tps = [psT.tile([128, 512], BF16, tag=f"tp{k}", name=f"tp{k}") for k in range(KCH)]
                    for j4 in range(4):
                        sc = nats[g4 * 4 + j4]
                        for k in range(KCH):
                            nc.tensor.transpose(
                                tps[k][:, j4 * 128:(j4 + 1) * 128],
                                sc[:, k * 128:(k + 1) * 128], eye_b[:])
                    for k in range(KCH):
                        nc.scalar.activation(
                            protoT[k][:, grp * 512:(grp + 1) * 512], tps[k][:],
                            AF.Copy, accum_out=sparts[k][:, grp:grp + 1])

                # Gram + entropy sums for the d-groups this chunk completed
                for dg in range(chunk * 2048 // DG, (chunk + 1) * 2048 // DG):
                    for cb in range(CBLK):
                        g = psG.tile([128, DG], F32, tag="g")
                        for ds in range(DG // 512):
                            for k in range(KCH):
                                d0 = dg * DG + ds * 512
                                nc.tensor.matmul(
                                    g[:, ds * 512:(ds + 1) * 512],
                                    protoT[k][:, cb * 128:(cb + 1) * 128],
                                    protoT[k][:, d0:d0 + 512],
                                    start=(k == 0), stop=(k == KCH - 1))
                        col = cb * NDG + dg
                        e_t = ewp.tile([128, DG], BF16, tag="e")
                        nc.scalar.activation(e_t[:], g[:], AF.Exp,
                                             accum_out=zparts[:, col:col + 1])
                        w_t = ewp.tile([128, DG], BF16, tag="w")
                        nc.vector.scalar_tensor_tensor(
                            out=w_t[:], in0=g[:], scalar=1.0, in1=e_t[:],
                            op0=ALU.mult, op1=ALU.mult,
                            accum_out=wparts[:, col:col + 1])

        # ---------- Phase R: rowsums via S matvec (PSUM banks now free) ----
        with tc.tile_pool(name="psumR", bufs=2, space="PSUM") as psR:
            for k in range(KCH):
                s_f = small.tile([128, 1], F32, tag="s_f")
                nc.vector.reduce_sum(s_f[:], sparts[k][:], axis=mybir.AxisListType.X)
                nc.vector.tensor_copy(s_bf[:, k:k + 1], s_f[:])
            for cb in range(CBLK):
                rs = psR.tile([128, 1], F32, tag="rs")
                for k in range(KCH):
                    nc.tensor.matmul(rs[:], protoT[k][:, cb * 128:(cb + 1) * 128],
                                     s_bf[:, k:k + 1],
                                     start=(k == 0), stop=(k == KCH - 1))
                nc.vector.tensor_copy(rowsums[:, cb:cb + 1], rs[:])

        # ---------- Tail: H rows, collective, colsum entropy ----------
        with tc.tile_pool(name="psumX", bufs=2, space="PSUM") as psX:
            z8 = small.tile([128, CBLK], F32, tag="z8")
            nc.vector.tensor_reduce(
                z8[:], zparts[:].rearrange("p (a b) -> p a b", b=NDG),
                axis=mybir.AxisListType.X, op=ALU.add)
            w8 = small.tile([128, CBLK], F32, tag="w8")
            nc.vector.tensor_reduce(
                w8[:], wparts[:].rearrange("p (a b) -> p a b", b=NDG),
                axis=mybir.AxisListType.X, op=ALU.add)
            lnz = small.tile([128, CBLK], F32, tag="lnz")
            nc.scalar.activation(lnz[:], z8[:], AF.Ln)
            rz = small.tile([128, CBLK], F32, tag="rz")
            nc.vector.reciprocal(rz[:], z8[:])
            h8 = small.tile([128, CBLK], F32, tag="h8")
            nc.vector.tensor_tensor(h8[:], w8[:], rz[:], ALU.mult)
            nc.vector.tensor_tensor(h8[:], lnz[:], h8[:], ALU.subtract)
            hsum = small.tile([128, 1], F32, tag="hsum")
            nc.vector.reduce_sum(hsum[:], h8[:], axis=mybir.AxisListType.X)
            hl_ps = psX.tile([1, 1], F32, tag="hl")
            nc.tensor.matmul(hl_ps[:], ones_col[:], hsum[:], start=True, stop=True)
            hloc = small.tile([1, 1], F32, tag="hloc")
            nc.vector.tensor_copy(hloc[:], hl_ps[:])

            # stage collective input: [rowsums(1024) | hloc]
            nc.gpsimd.dma_start(
                cc_in[0:RPC].rearrange("(b p) -> p b", p=128), rowsums[:])
            nc.gpsimd.dma_start(cc_in[RPC:RPC + 1], hloc[0, :])
            nc.gpsimd.collective_compute(
                "AllGather", ALU.bypass,
                replica_groups=[list(range(N_CORES))],
                ins=[cc_in[:]], outs=[cc_out[:]],
            )

            cc_view = cc_out.rearrange("(r c) -> r c", r=N_CORES)
            cs = small.tile([N_CORES, RPC], F32, tag="cs")
            nc.sync.dma_start(cs[:], cc_view[:, 0:RPC])
            stk = small.tile([N_CORES, 3], F32, tag="stk")
            nc.gpsimd.dma_start(stk[:, 2:3], cc_view[:, RPC:RPC + 1])

            m8 = small.tile([N_CORES, 1], F32, tag="m8")
            nc.vector.reduce_max(m8[:], cs[:], axis=mybir.AxisListType.X)
            mt_ps = psX.tile([1, N_CORES], F32, tag="mt")
            nc.tensor.transpose(mt_ps[:], m8[:], eye_f[0:N_CORES, 0:N_CORES])
            mx = small.tile([1, 1], F32, tag="mx")
            nc.vector.reduce_max(mx[:], mt_ps[:], axis=mybir.AxisListType.X)
            nmx = small.tile([1, 1], F32, tag="nmx")
            nc.vector.tensor_scalar(out=nmx[:], in0=mx[:], scalar1=-1.0,
                                    scalar2=None, op0=ALU.mult)
            nm_ps = psX.tile([N_CORES, 1], F32, tag="nm")
            nc.tensor.matmul(nm_ps[:], ones_row[:, 0:N_CORES], nmx[:],
                             start=True, stop=True)
            negm = small.tile([N_CORES, 1], F32, tag="negm")
            nc.vector.tensor_copy(negm[:], nm_ps[:])

            e_cs = small.tile([N_CORES, RPC], F32, tag="e_cs")
            nc.scalar.activation(e_cs[:], cs[:], AF.Exp, bias=negm[:],
                                 accum_out=stk[:, 0:1])
            wc = small.tile([N_CORES, RPC], F32, tag="wc")
            nc.vector.scalar_tensor_tensor(
                out=wc[:], in0=cs[:], scalar=1.0, in1=e_cs[:],
                op0=ALU.mult, op1=ALU.mult, accum_out=stk[:, 1:2])

            zw_ps = psX.tile([1, 3], F32, tag="zw")
            nc.tensor.matmul(zw_ps[:], ones_col[0:N_CORES, :], stk[:],
                             start=True, stop=True)
            fin = small.tile([1, 3], F32, tag="fin")
            nc.vector.tensor_copy(fin[:], zw_ps[:])

            # Hcol = ln Zc - Wc~/Zc + M ; loss = Hrowsum/C + Hcol
            lnzc = small.tile([1, 1], F32, tag="lnzc")
            nc.scalar.activation(lnzc[:], fin[:, 0:1], AF.Ln)
            rzc = small.tile([1, 1], F32, tag="rzc")
            nc.vector.reciprocal(rzc[:], fin[:, 0:1])
            acc = small.tile([1, 1], F32, tag="acc")
            nc.vector.tensor_tensor(acc[:], fin[:, 1:2], rzc[:], ALU.mult)
            nc.vector.tensor_tensor(acc[:], lnzc[:], acc[:], ALU.subtract)
            nc.vector.tensor_tensor(acc[:], acc[:], mx[:], ALU.add)
            hrow = small.tile([1, 1], F32, tag="hrow")
            nc.vector.tensor_scalar(out=hrow[:], in0=fin[:, 2:3],
                                    scalar1=1.0 / C, scalar2=None, op0=ALU.mult)
            nc.vector.tensor_tensor(acc[:], acc[:], hrow[:], ALU.add)
            nc.gpsimd.dma_start(loss_out[:], acc[0, :])


_CACHED = {}


def kernel(prototypes: np.ndarray) -> np.ndarray:
    assert prototypes.shape == (C, E) and prototypes.dtype == np.float32
    if "nc" not in _CACHED:
        _CACHED["nc"] = build_program(1)
    nc = _CACHED["nc"]
    in_maps = [
        {"prototypes": np.ascontiguousarray(
            np.concatenate([prototypes[i * RPC:], prototypes[:i * RPC]], axis=0))}
        for i in range(N_CORES)
    ]
    res = run_bass_kernel_spmd(nc, in_maps, list(range(N_CORES)))
    return np.float32(res.results[0]["loss"][0]).reshape(())


if __name__ == "__main__":
    rng = np.random.default_rng(0)
    p = rng.standard_normal((C, E), dtype=np.float32)
    print("loss:", kernel(p))


# revision 7
# speedup vs baseline: 2.6454x; 2.6454x over previous
"""Trainium2 Bass kernel for EntropyRegularizationLoss.

loss = mean_c H(softmax(G[c,:])) + H(softmax(G.sum(0)))  where
G = normalize(P) @ normalize(P).T,  P = prototypes [8192, 512] f32.

Distribution: 8 cores; core i receives P rotated by i*1024 rows so one
identical SPMD program always works on "rows 0:1024" as its Gram row
block. Row rotation permutes rows AND columns of G, which leaves both
the row entropies (row-local) and the colsum entropy (permutation
invariant) unchanged. One AllGather of [1024 rowsums | local H sum]
combines the cores.

Key algebraic moves:
  - normalization is folded into the operands, so the Gram comes out of
    the PE already normalized;
  - logits are cosine sims in [-1, 1] -> exp needs no max subtraction;
  - colsum of the (symmetric) full Gram == rowsum, and rowsum_c =
    p_c . S with S = sum_d p_d: a matvec instead of a 67M-elem pass;
  - ACT exp accumulates Z = sum e^x in-instruction (accum_out);
  - DVE tensor_tensor_reduce fuses w = x*e^x with W = sum w.
"""
import sys, json

sys.path.insert(0, "/opt/trn_rl_repo")

import numpy as np

import concourse.bass as bass
import concourse.tile as tile
from concourse import mybir
from concourse.bass_utils import run_bass_kernel_spmd

AF = mybir.ActivationFunctionType
ALU = mybir.AluOpType
F32 = mybir.dt.float32
BF16 = mybir.dt.bfloat16

N_CORES = 8
C, E = 8192, 512
RPC = C // N_CORES          # rows per core = 1024
CBLK = RPC // 128           # c-blocks per core = 8
KCH = E // 128              # contraction chunks = 4
NT = C // 128               # natural tiles = 64
DG = 1024                   # d-group width (2 PSUM banks)
NDG = C // DG               # d-groups = 4


def _split_sync_waits(bir: bytes, max_waits: int = 1) -> bytes:
    """walrus codegen rejects instructions with more than a few sem waits;
    split excess waits into chained Drain stubs on the same engine."""
    m = json.loads(bir)
    uid = [0]
    for fn in m["functions"]:
        for bb in fn["blocks"]:
            out_insts = []
            for inst in bb["instructions"]:
                si = inst.get("sync_info")
                ow = si.get("on_wait") if si else None
                if ow and len(ow) > max_waits:
                    chunks = [ow[i:i + max_waits] for i in range(0, len(ow), max_waits)]
                    for ch in chunks[:-1]:
                        uid[0] += 1
                        out_insts.append({
                            "debug": inst.get("debug"),
                            "engine": inst["engine"],
                            "ins": [],
                            "is_reset_sema": False,
                            "name": f"I-waitsplit-{uid[0]}",
                            "opcode": "Drain",
                            "outs": [],
                            "sync_info": {"on_update": [], "on_wait": ch},
                        })
                    si["on_wait"] = chunks[-1]
                out_insts.append(inst)
            bb["instructions"] = out_insts
    return json.dumps(m).encode()


def build_program(n_iters: int = 1):
    import ml_dtypes

    nc = bass.Bass("TRN2", target_bir_lowering=False, debug=False,
                   num_devices=N_CORES)
    proto = nc.dram_tensor("prototypes", [C, E], F32, kind="ExternalInput").ap()
    loss_out = nc.dram_tensor("loss", [1], F32, kind="ExternalOutput").ap()

    eye_bf = nc.inline_tensor(np.eye(128, dtype=ml_dtypes.bfloat16), name="eye_bf").ap()
    eye_f32 = nc.inline_tensor(np.eye(128, dtype=np.float32), name="eye_f32").ap()

    cc_in = nc.dram_tensor("cc_in", [RPC + 1], F32)
    cc_out = nc.dram_tensor("cc_out", [N_CORES * (RPC + 1)], F32,
                            addr_space="Shared")

    with tile.TileContext(nc) as tc:
        with tc.tile_pool(name="singles", bufs=1) as singles:
            ones_col = singles.tile([128, 1], F32)   # lhsT for partition sums
            nc.vector.memset(ones_col, 1.0)
            ones_row = singles.tile([1, 128], F32)   # lhsT for partition bcast
            nc.vector.memset(ones_row, 1.0)
            eye_b = singles.tile([128, 128], BF16)
            nc.sync.dma_start(eye_b[:], eye_bf[:])
            eye_f = singles.tile([128, 128], F32)
            nc.sync.dma_start(eye_f[:], eye_f32[:])

            for _ in range(n_iters):
                _one_iter(nc, tc, proto, loss_out, cc_in, cc_out,
                          ones_col, ones_row, eye_b, eye_f)

    orig = nc.to_json_bytes
    nc.to_json_bytes = lambda: _split_sync_waits(orig())
    return nc


def _one_iter(nc, tc, proto, loss_out, cc_in, cc_out,
              ones_col, ones_row, eye_b, eye_f):
    with (
        tc.tile_pool(name="persist", bufs=1) as persist,
        tc.tile_pool(name="nat", bufs=4) as natp,
        tc.tile_pool(name="scaled", bufs=4) as scp,
        tc.tile_pool(name="scratch", bufs=2) as scr,
        tc.tile_pool(name="small", bufs=2) as small,
    ):
        # protoT[k]: [128, 8192] bf16, k-th 128-slice of E on partitions
        protoT = [persist.tile([128, C], BF16, tag=f"pt{k}", name=f"pt{k}") for k in range(KCH)]
        normsq = persist.tile([128, NT], F32, tag="normsq")
        invn = persist.tile([128, NT], F32, tag="invn")
        sparts = [persist.tile([128, NT // 4], F32, tag=f"sp{k}", name=f"sp{k}") for k in range(KCH)]
        zparts = persist.tile([128, CBLK * NDG], F32, tag="zparts")
        wparts = persist.tile([128, CBLK * NDG], F32, tag="wparts")
        rowsums = persist.tile([128, CBLK], F32, tag="rowsums")
        s_bf = persist.tile([128, KCH], BF16, tag="s_bf")

        # Wavefront over chunks of 16 natural tiles: load+normalize+transpose
        # chunk c, then immediately emit the Gram d-groups that chunk c
        # unlocked (dg = 2c, 2c+1 vs every cb; the lhsT c-blocks all live in
        # chunk 0's columns). This keeps PE's serial instruction stream dense:
        # transposes(c) | grams(c) | transposes(c+1) | grams(c+1) | ...
        # psT (4 banks) and psG (4 banks, nested) get disjoint PSUM banks.
        CHUNK = 16
        with (
            tc.tile_pool(name="psumT", bufs=1, space="PSUM") as psT,
            tc.tile_pool(name="psumG", bufs=2, space="PSUM") as psG,
            tc.tile_pool(name="ew", bufs=3) as ewp,
        ):
            for chunk in range(NT // CHUNK):
                nats = []
                for j in range(CHUNK):
                    t = chunk * CHUNK + j
                    nat = natp.tile([128, E], F32, tag=f"nat{j % 4}")
                    nc.sync.dma_start(nat[:], proto[t * 128:(t + 1) * 128, :])
                    sq = scr.tile([128, E], F32, tag="sq")
                    nc.vector.scalar_tensor_tensor(
                        out=sq[:], in0=nat[:], scalar=1.0, in1=nat[:],
                        op0=ALU.mult, op1=ALU.mult,
                        accum_out=normsq[:, t:t + 1])
                    nats.append(nat)
                c16 = slice(chunk * CHUNK, (chunk + 1) * CHUNK)
                snorm = small.tile([128, CHUNK], F32, tag="snorm")
                nc.scalar.activation(snorm[:], normsq[:, c16], AF.Sqrt)
                r0 = small.tile([128, CHUNK], F32, tag="r0")
                nc.vector.reciprocal(r0[:], snorm[:])
                # one Newton step for rsqrt: y = r0*(1.5 - 0.5*x*r0^2)
                t1 = small.tile([128, CHUNK], F32, tag="t1")
                nc.vector.tensor_tensor(t1[:], r0[:], r0[:], ALU.mult)
                nc.vector.tensor_tensor(t1[:], t1[:], normsq[:, c16], ALU.mult)
                nc.vector.tensor_scalar(out=t1[:], in0=t1[:], scalar1=-0.5,
                                        scalar2=1.5, op0=ALU.mult, op1=ALU.add)
                nc.vector.tensor_tensor(invn[:, c16], r0[:], t1[:], ALU.mult)

                for j in range(CHUNK):
                    t = chunk * CHUNK + j
                    sc = scp.tile([128, E], BF16, tag=f"sc{j % 4}")
                    nc.scalar.activation(sc[:], nats[j][:], AF.Copy,
                                         scale=invn[:, t:t + 1])
                    nats[j] = sc
                # transpose groups of 4 tiles -> protoT[k][:, grp*512:+512]
                for g4 in range(CHUNK // 4):
                    grp = chunk * 4 + g4
                    tps = [psT.tile([128, 512], BF16, tag=f"tp{k}", name=f"tp{k}") for k in range(KCH)]
                    for j4 in range(4):
                        sc = nats[g4 * 4 + j4]
                        for k in range(KCH):
                            nc.tensor.transpose(
                                tps[k][:, j4 * 128:(j4 + 1) * 128],
                                sc[:, k * 128:(k + 1) * 128], eye_b[:])
                    for k in range(KCH):
                        nc.scalar.activation(
                            protoT[k][:, grp * 512:(grp + 1) * 512], tps[k][:],
                            AF.Copy, accum_out=sparts[k][:, grp:grp + 1])

                # Gram + entropy sums for the d-groups this chunk completed
                for dg in range(chunk * 2048 // DG, (chunk + 1) * 2048 // DG):
                    for cb in range(CBLK):
                        g = psG.tile([128, DG], F32, tag="g")
                        for ds in range(DG // 512):
                            for k in range(KCH):
                                d0 = dg * DG + ds * 512
                                nc.tensor.matmul(
                                    g[:, ds * 512:(ds + 1) * 512],
                                    protoT[k][:, cb * 128:(cb + 1) * 128],
                                    protoT[k][:, d0:d0 + 512],
                                    start=(k == 0), stop=(k == KCH - 1))
                        col = cb * NDG + dg
                        e_t = ewp.tile([128, DG], BF16, tag="e")
                        nc.scalar.activation(e_t[:], g[:], AF.Exp,
                                             accum_out=zparts[:, col:col + 1])
                        w_t = ewp.tile([128, DG], BF16, tag="w")
                        nc.vector.scalar_tensor_tensor(
                            out=w_t[:], in0=g[:], scalar=1.0, in1=e_t[:],
                            op0=ALU.mult, op1=ALU.mult,
                            accum_out=wparts[:, col:col + 1])

        # ---------- Phase R: rowsums via S matvec (PSUM banks now free) ----
        with tc.tile_pool(name="psumR", bufs=2, space="PSUM") as psR:
            for k in range(KCH):
                s_f = small.tile([128, 1], F32, tag="s_f")
                nc.vector.reduce_sum(s_f[:], sparts[k][:], axis=mybir.AxisListType.X)
                nc.vector.tensor_copy(s_bf[:, k:k + 1], s_f[:])
            for cb in range(CBLK):
                rs = psR.tile([128, 1], F32, tag="rs")
                for k in range(KCH):
                    nc.tensor.matmul(rs[:], protoT[k][:, cb * 128:(cb + 1) * 128],
                                     s_bf[:, k:k + 1],
                                     start=(k == 0), stop=(k == KCH - 1))
                nc.vector.tensor_copy(rowsums[:, cb:cb + 1], rs[:])

        # ---------- Tail: H rows, collective, colsum entropy ----------
        with tc.tile_pool(name="psumX", bufs=2, space="PSUM") as psX:
            z8 = small.tile([128, CBLK], F32, tag="z8")
            nc.vector.tensor_reduce(
                z8[:], zparts[:].rearrange("p (a b) -> p a b", b=NDG),
                axis=mybir.AxisListType.X, op=ALU.add)
            w8 = small.tile([128, CBLK], F32, tag="w8")
            nc.vector.tensor_reduce(
                w8[:], wparts[:].rearrange("p (a b) -> p a b", b=NDG),
                axis=mybir.AxisListType.X, op=ALU.add)
            lnz = small.tile([128, CBLK], F32, tag="lnz")
            nc.scalar.activation(lnz[:], z8[:], AF.Ln)
            rz = small.tile([128, CBLK], F32, tag="rz")
            nc.vector.reciprocal(rz[:], z8[:])
            h8 = small.tile([128, CBLK], F32, tag="h8")
            nc.vector.tensor_tensor(h8[:], w8[:], rz[:], ALU.mult)
            nc.vector.tensor_tensor(h8[:], lnz[:], h8[:], ALU.subtract)
            hsum = small.tile([128, 1], F32, tag="hsum")
            nc.vector.reduce_sum(hsum[:], h8[:], axis=mybir.AxisListType.X)
            hl_ps = psX.tile([1, 1], F32, tag="hl")
            nc.tensor.matmul(hl_ps[:], ones_col[:], hsum[:], start=True, stop=True)
            hloc = small.tile([1, 1], F32, tag="hloc")
            nc.vector.tensor_copy(hloc[:], hl_ps[:])

            # stage collective input: [rowsums(1024) | hloc]
            nc.gpsimd.dma_start(
                cc_in[0:RPC].rearrange("(b p) -> p b", p=128), rowsums[:])
            nc.gpsimd.dma_start(cc_in[RPC:RPC + 1], hloc[0, :])
            nc.gpsimd.collective_compute(
                "AllGather", ALU.bypass,
                replica_groups=[list(range(N_CORES))],
                ins=[cc_in[:]], outs=[cc_out[:]],
            )

            cc_view = cc_out.rearrange("(r c) -> r c", r=N_CORES)
            cs = small.tile([N_CORES, RPC], F32, tag="cs")
            nc.sync.dma_start(cs[:], cc_view[:, 0:RPC])
            stk = small.tile([N_CORES, 3], F32, tag="stk")
            nc.gpsimd.dma_start(stk[:, 2:3], cc_view[:, RPC:RPC + 1])

            m8 = small.tile([N_CORES, 1], F32, tag="m8")
            nc.vector.reduce_max(m8[:], cs[:], axis=mybir.AxisListType.X)
            mt_ps = psX.tile([1, N_CORES], F32, tag="mt")
            nc.tensor.transpose(mt_ps[:], m8[:], eye_f[0:N_CORES, 0:N_CORES])
            mx = small.tile([1, 1], F32, tag="mx")
            nc.vector.reduce_max(mx[:], mt_ps[:], axis=mybir.AxisListType.X)
            nmx = small.tile([1, 1], F32, tag="nmx")
            nc.vector.tensor_scalar(out=nmx[:], in0=mx[:], scalar1=-1.0,
                                    scalar2=None, op0=ALU.mult)
            nm_ps = psX.tile([N_CORES, 1], F32, tag="nm")
            nc.tensor.matmul(nm_ps[:], ones_row[:, 0:N_CORES], nmx[:],
                             start=True, stop=True)
            negm = small.tile([N_CORES, 1], F32, tag="negm")
            nc.vector.tensor_copy(negm[:], nm_ps[:])

            e_cs = small.tile([N_CORES, RPC], F32, tag="e_cs")
            nc.scalar.activation(e_cs[:], cs[:], AF.Exp, bias=negm[:],
                                 accum_out=stk[:, 0:1])
            wc = small.tile([N_CORES, RPC], F32, tag="wc")
            nc.vector.scalar_tensor_tensor(
                out=wc[:], in0=cs[:], scalar=1.0, in1=e_cs[:],
                op0=ALU.mult, op1=ALU.mult, accum_out=stk[:, 1:2])

            zw_ps = psX.tile([1, 3], F32, tag="zw")
            nc.tensor.matmul(zw_ps[:], ones_col[0:N_CORES, :], stk[:],
                             start=True, stop=True)
            fin = small.tile([1, 3], F32, tag="fin")
            nc.vector.tensor_copy(fin[:], zw_ps[:])

            # Hcol = ln Zc - Wc~/Zc + M ; loss = Hrowsum/C + Hcol
            lnzc = small.tile([1, 1], F32, tag="lnzc")
            nc.scalar.activation(lnzc[:], fin[:, 0:1], AF.Ln)
            rzc = small.tile([1, 1], F32, tag="rzc")
            nc.vector.reciprocal(rzc[:], fin[:, 0:1])
            acc = small.tile([1, 1], F32, tag="acc")
            nc.vector.tensor_tensor(acc[:], fin[:, 1:2], rzc[:], ALU.mult)
            nc.vector.tensor_tensor(acc[:], lnzc[:], acc[:], ALU.subtract)
            nc.vector.tensor_tensor(acc[:], acc[:], mx[:], ALU.add)
            hrow = small.tile([1, 1], F32, tag="hrow")
            nc.vector.tensor_scalar(out=hrow[:], in0=fin[:, 2:3],
                                    scalar1=1.0 / C, scalar2=None, op0=ALU.mult)
            nc.vector.tensor_tensor(acc[:], acc[:], hrow[:], ALU.add)
            nc.gpsimd.dma_start(loss_out[:], acc[0, :])


_CACHED = {}


def kernel(prototypes: np.ndarray) -> np.ndarray:
    assert prototypes.shape == (C, E) and prototypes.dtype == np.float32
    if "nc" not in _CACHED:
        _CACHED["nc"] = build_program(1)
    nc = _CACHED["nc"]
    in_maps = [
        {"prototypes": np.ascontiguousarray(
            np.concatenate([prototypes[i * RPC:], prototypes[:i * RPC]], axis=0))}
        for i in range(N_CORES)
    ]
    res = run_bass_kernel_spmd(nc, in_maps, list(range(N_CORES)))
    return np.float32(res.results[0]["loss"][0]).reshape(())


if __name__ == "__main__":
    rng = np.random.default_rng(0)
    p = rng.standard_normal((C, E), dtype=np.float32)
    print("loss:", kernel(p))


# revision 8
# speedup vs baseline: 2.6577x; 1.0046x over previous
"""Trainium2 Bass kernel for EntropyRegularizationLoss.

loss = mean_c H(softmax(G[c,:])) + H(softmax(G.sum(0)))  where
G = normalize(P) @ normalize(P).T,  P = prototypes [8192, 512] f32.

Distribution: 8 cores; core i receives P rotated by i*1024 rows so one
identical SPMD program always works on "rows 0:1024" as its Gram row
block. Row rotation permutes rows AND columns of G, which leaves both
the row entropies (row-local) and the colsum entropy (permutation
invariant) unchanged. One AllGather of [1024 rowsums | local H sum]
combines the cores.

Key algebraic moves:
  - normalization is folded into the operands, so the Gram comes out of
    the PE already normalized;
  - logits are cosine sims in [-1, 1] -> exp needs no max subtraction;
  - colsum of the (symmetric) full Gram == rowsum, and rowsum_c =
    p_c . S with S = sum_d p_d: a matvec instead of a 67M-elem pass;
  - ACT exp accumulates Z = sum e^x in-instruction (accum_out);
  - DVE tensor_tensor_reduce fuses w = x*e^x with W = sum w.
"""
import sys, json

sys.path.insert(0, "/opt/trn_rl_repo")

import numpy as np

import concourse.bass as bass
import concourse.tile as tile
from concourse import mybir
from concourse.bass_utils import run_bass_kernel_spmd

AF = mybir.ActivationFunctionType
ALU = mybir.AluOpType
F32 = mybir.dt.float32
BF16 = mybir.dt.bfloat16

N_CORES = 8
C, E = 8192, 512
RPC = C // N_CORES          # rows per core = 1024
CBLK = RPC // 128           # c-blocks per core = 8
KCH = E // 128              # contraction chunks = 4
NT = C // 128               # natural tiles = 64
DG = 1024                   # d-group width (2 PSUM banks)
NDG = C // DG               # d-groups = 4


def _split_sync_waits(bir: bytes, max_waits: int = 1) -> bytes:
    """walrus codegen rejects instructions with more than a few sem waits;
    split excess waits into chained Drain stubs on the same engine."""
    m = json.loads(bir)
    uid = [0]
    for fn in m["functions"]:
        for bb in fn["blocks"]:
            out_insts = []
            for inst in bb["instructions"]:
                si = inst.get("sync_info")
                ow = si.get("on_wait") if si else None
                if ow and len(ow) > max_waits:
                    chunks = [ow[i:i + max_waits] for i in range(0, len(ow), max_waits)]
                    for ch in chunks[:-1]:
                        uid[0] += 1
                        out_insts.append({
                            "debug": inst.get("debug"),
                            "engine": inst["engine"],
                            "ins": [],
                            "is_reset_sema": False,
                            "name": f"I-waitsplit-{uid[0]}",
                            "opcode": "Drain",
                            "outs": [],
                            "sync_info": {"on_update": [], "on_wait": ch},
                        })
                    si["on_wait"] = chunks[-1]
                out_insts.append(inst)
            bb["instructions"] = out_insts
    return json.dumps(m).encode()


def build_program(n_iters: int = 1):
    import ml_dtypes

    nc = bass.Bass("TRN2", target_bir_lowering=False, debug=False,
                   num_devices=N_CORES)
    proto = nc.dram_tensor("prototypes", [C, E], F32, kind="ExternalInput").ap()
    loss_out = nc.dram_tensor("loss", [1], F32, kind="ExternalOutput").ap()

    eye_bf = nc.inline_tensor(np.eye(128, dtype=ml_dtypes.bfloat16), name="eye_bf").ap()
    eye_f32 = nc.inline_tensor(np.eye(128, dtype=np.float32), name="eye_f32").ap()

    cc_in = nc.dram_tensor("cc_in", [RPC + 1], F32)
    cc_out = nc.dram_tensor("cc_out", [N_CORES * (RPC + 1)], F32,
                            addr_space="Shared")

    with tile.TileContext(nc) as tc:
        with tc.tile_pool(name="singles", bufs=1) as singles:
            ones_col = singles.tile([128, 1], F32)   # lhsT for partition sums
            nc.vector.memset(ones_col, 1.0)
            ones_row = singles.tile([1, 128], F32)   # lhsT for partition bcast
            nc.vector.memset(ones_row, 1.0)
            eye_b = singles.tile([128, 128], BF16)
            nc.sync.dma_start(eye_b[:], eye_bf[:])
            eye_f = singles.tile([128, 128], F32)
            nc.sync.dma_start(eye_f[:], eye_f32[:])

            for _ in range(n_iters):
                _one_iter(nc, tc, proto, loss_out, cc_in, cc_out,
                          ones_col, ones_row, eye_b, eye_f)

    orig = nc.to_json_bytes
    nc.to_json_bytes = lambda: _split_sync_waits(orig())
    return nc


def _one_iter(nc, tc, proto, loss_out, cc_in, cc_out,
              ones_col, ones_row, eye_b, eye_f):
    with (
        tc.tile_pool(name="persist", bufs=1) as persist,
        tc.tile_pool(name="nat", bufs=4) as natp,
        tc.tile_pool(name="scaled", bufs=4) as scp,
        tc.tile_pool(name="scratch", bufs=2) as scr,
        tc.tile_pool(name="small", bufs=2) as small,
    ):
        # protoT[k]: [128, 8192] bf16, k-th 128-slice of E on partitions
        protoT = [persist.tile([128, C], BF16, tag=f"pt{k}", name=f"pt{k}") for k in range(KCH)]
        normsq = persist.tile([128, NT], F32, tag="normsq")
        invn = persist.tile([128, NT], F32, tag="invn")
        sparts = [persist.tile([128, NT // 4], F32, tag=f"sp{k}", name=f"sp{k}") for k in range(KCH)]
        zparts = persist.tile([128, CBLK * NDG], F32, tag="zparts")
        wparts = persist.tile([128, CBLK * NDG], F32, tag="wparts")
        rowsums = persist.tile([128, CBLK], F32, tag="rowsums")
        s_bf = persist.tile([128, KCH], BF16, tag="s_bf")

        # Wavefront over chunks of 16 natural tiles: load+normalize+transpose
        # chunk c, then immediately emit the Gram d-groups that chunk c
        # unlocked (dg = 2c, 2c+1 vs every cb; the lhsT c-blocks all live in
        # chunk 0's columns). This keeps PE's serial instruction stream dense:
        # transposes(c) | grams(c) | transposes(c+1) | grams(c+1) | ...
        # psT (4 banks) and psG (4 banks, nested) get disjoint PSUM banks.
        CHUNK = 16
        with (
            tc.tile_pool(name="psumT", bufs=1, space="PSUM") as psT,
            tc.tile_pool(name="psumG", bufs=2, space="PSUM") as psG,
            tc.tile_pool(name="ew", bufs=3) as ewp,
        ):
            for chunk in range(NT // CHUNK):
                nats = []
                for j in range(CHUNK):
                    t = chunk * CHUNK + j
                    nat = natp.tile([128, E], F32, tag=f"nat{j % 4}")
                    nc.sync.dma_start(nat[:], proto[t * 128:(t + 1) * 128, :])
                    sq = scr.tile([128, E], F32, tag="sq")
                    nc.vector.scalar_tensor_tensor(
                        out=sq[:], in0=nat[:], scalar=1.0, in1=nat[:],
                        op0=ALU.mult, op1=ALU.mult,
                        accum_out=normsq[:, t:t + 1])
                    nats.append(nat)
                c16 = slice(chunk * CHUNK, (chunk + 1) * CHUNK)
                snorm = small.tile([128, CHUNK], F32, tag="snorm")
                nc.scalar.activation(snorm[:], normsq[:, c16], AF.Sqrt)
                r0 = small.tile([128, CHUNK], F32, tag="r0")
                nc.vector.reciprocal(r0[:], snorm[:])
                # one Newton step for rsqrt: y = r0*(1.5 - 0.5*x*r0^2)
                t1 = small.tile([128, CHUNK], F32, tag="t1")
                nc.vector.tensor_tensor(t1[:], r0[:], r0[:], ALU.mult)
                nc.vector.tensor_tensor(t1[:], t1[:], normsq[:, c16], ALU.mult)
                nc.vector.tensor_scalar(out=t1[:], in0=t1[:], scalar1=-0.5,
                                        scalar2=1.5, op0=ALU.mult, op1=ALU.add)
                nc.vector.tensor_tensor(invn[:, c16], r0[:], t1[:], ALU.mult)

                for j in range(CHUNK):
                    t = chunk * CHUNK + j
                    sc = scp.tile([128, E], BF16, tag=f"sc{j % 4}")
                    nc.vector.tensor_scalar(out=sc[:], in0=nats[j][:],
                                            scalar1=invn[:, t:t + 1],
                                            scalar2=None, op0=ALU.mult)
                    nats[j] = sc
                # transpose groups of 4 tiles -> protoT[k][:, grp*512:+512]
                for g4 in range(CHUNK // 4):
                    grp = chunk * 4 + g4
                    tps = [psT.tile([128, 512], BF16, tag=f"tp{k}", name=f"tp{k}") for k in range(KCH)]
                    for j4 in range(4):
                        sc = nats[g4 * 4 + j4]
                        for k in range(KCH):
                            nc.tensor.transpose(
                                tps[k][:, j4 * 128:(j4 + 1) * 128],
                                sc[:, k * 128:(k + 1) * 128], eye_b[:])
                    for k in range(KCH):
                        nc.scalar.activation(
                            protoT[k][:, grp * 512:(grp + 1) * 512], tps[k][:],
                            AF.Copy, accum_out=sparts[k][:, grp:grp + 1])

                # Gram + entropy sums for the d-groups this chunk completed
                for dg in range(chunk * 2048 // DG, (chunk + 1) * 2048 // DG):
                    for cb in range(CBLK):
                        g = psG.tile([128, DG], F32, tag="g")
                        for ds in range(DG // 512):
                            for k in range(KCH):
                                d0 = dg * DG + ds * 512
                                nc.tensor.matmul(
                                    g[:, ds * 512:(ds + 1) * 512],
                                    protoT[k][:, cb * 128:(cb + 1) * 128],
                                    protoT[k][:, d0:d0 + 512],
                                    start=(k == 0), stop=(k == KCH - 1))
                        col = cb * NDG + dg
                        e_t = ewp.tile([128, DG], BF16, tag="e")
                        nc.scalar.activation(e_t[:], g[:], AF.Exp,
                                             accum_out=zparts[:, col:col + 1])
                        w_t = ewp.tile([128, DG], BF16, tag="w")
                        nc.vector.scalar_tensor_tensor(
                            out=w_t[:], in0=g[:], scalar=1.0, in1=e_t[:],
                            op0=ALU.mult, op1=ALU.mult,
                            accum_out=wparts[:, col:col + 1])

        # ---------- Phase R: rowsums via S matvec (PSUM banks now free) ----
        with tc.tile_pool(name="psumR", bufs=2, space="PSUM") as psR:
            for k in range(KCH):
                s_f = small.tile([128, 1], F32, tag="s_f")
                nc.vector.reduce_sum(s_f[:], sparts[k][:], axis=mybir.AxisListType.X)
                nc.vector.tensor_copy(s_bf[:, k:k + 1], s_f[:])
            for cb in range(CBLK):
                rs = psR.tile([128, 1], F32, tag="rs")
                for k in range(KCH):
                    nc.tensor.matmul(rs[:], protoT[k][:, cb * 128:(cb + 1) * 128],
                                     s_bf[:, k:k + 1],
                                     start=(k == 0), stop=(k == KCH - 1))
                nc.vector.tensor_copy(rowsums[:, cb:cb + 1], rs[:])

        # ---------- Tail: H rows, collective, colsum entropy ----------
        with tc.tile_pool(name="psumX", bufs=2, space="PSUM") as psX:
            z8 = small.tile([128, CBLK], F32, tag="z8")
            nc.vector.tensor_reduce(
                z8[:], zparts[:].rearrange("p (a b) -> p a b", b=NDG),
                axis=mybir.AxisListType.X, op=ALU.add)
            w8 = small.tile([128, CBLK], F32, tag="w8")
            nc.vector.tensor_reduce(
                w8[:], wparts[:].rearrange("p (a b) -> p a b", b=NDG),
                axis=mybir.AxisListType.X, op=ALU.add)
            lnz = small.tile([128, CBLK], F32, tag="lnz")
            nc.scalar.activation(lnz[:], z8[:], AF.Ln)
            rz = small.tile([128, CBLK], F32, tag="rz")
            nc.vector.reciprocal(rz[:], z8[:])
            h8 = small.tile([128, CBLK], F32, tag="h8")
            nc.vector.tensor_tensor(h8[:], w8[:], rz[:], ALU.mult)
            nc.vector.tensor_tensor(h8[:], lnz[:], h8[:], ALU.subtract)
            hsum = small.tile([128, 1], F32, tag="hsum")
            nc.vector.reduce_sum(hsum[:], h8[:], axis=mybir.AxisListType.X)
            hl_ps = psX.tile([1, 1], F32, tag="hl")
            nc.tensor.matmul(hl_ps[:], ones_col[:], hsum[:], start=True, stop=True)
            hloc = small.tile([1, 1], F32, tag="hloc")
            nc.vector.tensor_copy(hloc[:], hl_ps[:])

            # stage collective input: [rowsums(1024) | hloc]
            nc.gpsimd.dma_start(
                cc_in[0:RPC].rearrange("(b p) -> p b", p=128), rowsums[:])
            nc.gpsimd.dma_start(cc_in[RPC:RPC + 1], hloc[0, :])
            nc.gpsimd.collective_compute(
                "AllGather", ALU.bypass,
                replica_groups=[list(range(N_CORES))],
                ins=[cc_in[:]], outs=[cc_out[:]],
            )

            cc_view = cc_out.rearrange("(r c) -> r c", r=N_CORES)
            cs = small.tile([N_CORES, RPC], F32, tag="cs")
            nc.sync.dma_start(cs[:], cc_view[:, 0:RPC])
            stk = small.tile([N_CORES, 3], F32, tag="stk")
            nc.gpsimd.dma_start(stk[:, 2:3], cc_view[:, RPC:RPC + 1])

            m8 = small.tile([N_CORES, 1], F32, tag="m8")
            nc.vector.reduce_max(m8[:], cs[:], axis=mybir.AxisListType.X)
            mt_ps = psX.tile([1, N_CORES], F32, tag="mt")
            nc.tensor.transpose(mt_ps[:], m8[:], eye_f[0:N_CORES, 0:N_CORES])
            mx = small.tile([1, 1], F32, tag="mx")
            nc.vector.reduce_max(mx[:], mt_ps[:], axis=mybir.AxisListType.X)
            nmx = small.tile([1, 1], F32, tag="nmx")
            nc.vector.tensor_scalar(out=nmx[:], in0=mx[:], scalar1=-1.0,
                                    scalar2=None, op0=ALU.mult)
            nm_ps = psX.tile([N_CORES, 1], F32, tag="nm")
            nc.tensor.matmul(nm_ps[:], ones_row[:, 0:N_CORES], nmx[:],
                             start=True, stop=True)
            negm = small.tile([N_CORES, 1], F32, tag="negm")
            nc.vector.tensor_copy(negm[:], nm_ps[:])

            e_cs = small.tile([N_CORES, RPC], F32, tag="e_cs")
            nc.scalar.activation(e_cs[:], cs[:], AF.Exp, bias=negm[:],
                                 accum_out=stk[:, 0:1])
            wc = small.tile([N_CORES, RPC], F32, tag="wc")
            nc.vector.scalar_tensor_tensor(
                out=wc[:], in0=cs[:], scalar=1.0, in1=e_cs[:],
                op0=ALU.mult, op1=ALU.mult, accum_out=stk[:, 1:2])

            zw_ps = psX.tile([1, 3], F32, tag="zw")
            nc.tensor.matmul(zw_ps[:], ones_col[0:N_CORES, :], stk[:],
                             start=True, stop=True)
            fin = small.tile([1, 3], F32, tag="fin")
            nc.vector.tensor_copy(fin[:], zw_ps[:])

            # Hcol = ln Zc - Wc~/Zc + M ; loss = Hrowsum/C + Hcol
            lnzc = small.tile([1, 1], F32, tag="lnzc")
            nc.scalar.activation(lnzc[:], fin[:, 0:1], AF.Ln)
            rzc = small.tile([1, 1], F32, tag="rzc")
            nc.vector.reciprocal(rzc[:], fin[:, 0:1])
            acc = small.tile([1, 1], F32, tag="acc")
            nc.vector.tensor_tensor(acc[:], fin[:, 1:2], rzc[:], ALU.mult)
            nc.vector.tensor_tensor(acc[:], lnzc[:], acc[:], ALU.subtract)
            nc.vector.tensor_tensor(acc[:], acc[:], mx[:], ALU.add)
            hrow = small.tile([1, 1], F32, tag="hrow")
            nc.vector.tensor_scalar(out=hrow[:], in0=fin[:, 2:3],
                                    scalar1=1.0 / C, scalar2=None, op0=ALU.mult)
            nc.vector.tensor_tensor(acc[:], acc[:], hrow[:], ALU.add)
            nc.gpsimd.dma_start(loss_out[:], acc[0, :])


_CACHED = {}


def kernel(prototypes: np.ndarray) -> np.ndarray:
    assert prototypes.shape == (C, E) and prototypes.dtype == np.float32
    if "nc" not in _CACHED:
        _CACHED["nc"] = build_program(1)
    nc = _CACHED["nc"]
    in_maps = [
        {"prototypes": np.ascontiguousarray(
            np.concatenate([prototypes[i * RPC:], prototypes[:i * RPC]], axis=0))}
        for i in range(N_CORES)
    ]
    res = run_bass_kernel_spmd(nc, in_maps, list(range(N_CORES)))
    return np.float32(res.results[0]["loss"][0]).reshape(())


if __name__ == "__main__":
    rng = np.random.default_rng(0)
    p = rng.standard_normal((C, E), dtype=np.float32)
    print("loss:", kernel(p))
